# revision 6
# baseline (speedup 1.0000x reference)
"""JointCCSA loss kernel — v4: circulant band tiling, shared-X operands.

Each core c owns row-band c (512 rows) and covers pair-blocks (c, c+d) for
d=0..3 plus half of (c, c+4): 18 real [128x512] tiles, no dummy slots.
lhs and rhs both read from ONE shared fp8 X-panel buffer (plain X, not -2X);
the -2 factor moves into the ACT sqrt's scale.  sq_j enters the PSUM via a
rank-4 fp8 DoubleRow matmul (hi/lo/lo2 split), sq_i via the ACT bias.
Per-core input drops 2.35MB -> ~1.45MB and is DMA'd in need-order so the
first group (self-band, needs one 256KB panel) starts at ~6.5us.
Postamble semaphore sweep shrunk via walrus --max-sem-num.
"""

import numpy as np
import ml_dtypes
from contextlib import ExitStack

import concourse.bass as bass
import concourse.tile as tile
from concourse import mybir
from concourse.vector_clock import ScopedClock
from concourse.bass_utils import run_bass_kernel_spmd
import concourse.bass_utils as _bass_utils

BS = 4096
D = 512
NCORES = 8
PW = 512
NSLOT = 18
NGRP = 5
GSIZE = [4, 4, 4, 4, 2]
GSTART = [0, 4, 8, 12, 16]
C0 = 0.0625
SHIFT = 32.0
WARMUP = 18
F8 = ml_dtypes.float8_e4m3
BF16 = ml_dtypes.bfloat16

# --- walrus build patches (same as v3) -------------------------------------
def _patched_drain_and_barrier(self, tick_clock, wait_clock):
    nc = self.nc
    coll = nc.sync.nop(nofuse=True, hint="drain_wait_collector")
    wait_clock.add_sem_waits(coll.ins, ScopedClock({None: tick_clock.global_clock}))
    si = coll.ins.sync_info
    waits = list(si.on_wait) if si is not None else []
    if len(waits) > 1:
        si.on_wait = [waits[0]]
        for w in waits[1:]:
            n = nc.sync.nop(nofuse=True, hint="drain_wait_extra")
            n.ins.sync_info = mybir.SyncInfo(on_wait=[w], on_update=[])
    nc.sync.drain()
    nc.all_engine_barrier()
    assert self.sems is not None
    popped = nc._tile_sem_poison_stack.pop()
    assert popped is self._sem_poison
    nc.clear_and_free_semaphores(list(self.sems.allocated().values()))
    nc.all_engine_barrier()


tile.TileContext._drain_and_barrier = _patched_drain_and_barrier


def _split_waits(nc, maxw=1):
    for fn in nc.m.functions:
        for blk in fn.blocks:
            newlist = []
            for inst in blk.instructions:
                si = getattr(inst, "sync_info", None)
                if si is not None and len(si.on_wait) > maxw:
                    waits = list(si.on_wait)
                    for i, w in enumerate(waits[maxw:]):
                        nop = mybir.InstNoOp(
                            name=f"{inst.name}-wsplit{i}",
                            sync_info=mybir.SyncInfo(on_wait=[w], on_update=[]),
                            bass_nofuse=True,
                            engine=inst.engine,
                        )
                        nc.register_instruction(nop)
                        newlist.append(nop)
                    si.on_wait = waits[:maxw]
                newlist.append(inst)
            blk.instructions[:] = newlist
# ---------------------------------------------------------------------------


def tiles_for_core(c):
    """18 tiles of (rows_start, panel, ordered), groups (4,4,4,4,2)."""
    out = []
    for d in range(4):
        p = (c + d) % 8
        for k in range(4):
            out.append((c * 512 + k * 128, p, d == 0))
    if c < 4:
        for k in range(2):
            out.append((c * 512 + k * 128, c + 4, False))
    else:
        for k in range(2):
            out.append(((c - 4) * 512 + (2 + k) * 128, c, False))
    return out


# slot -> (lhs source, lhs col offset).  lhs source 0 = xpan slot 0 (own
# band), 1 = xd4.  rhs panel-slot for slot t: RSLOT[t].
LHS = [(0, k % 4) for k in range(16)] + [(1, 0), (1, 1)]
RSLOT = [0] * 4 + [1] * 4 + [2] * 4 + [3] * 4 + [4] * 2

_NC_CACHE = {}


def build_program():
    if "nc" in _NC_CACHE:
        return _NC_CACHE["nc"]
    f32 = mybir.dt.float32
    bf16 = mybir.dt.bfloat16
    f8 = mybir.dt.float8e4
    DR = mybir.MatmulPerfMode.DoubleRow

    nc = bass.Bass()
    # Warmup source initialized pre-TileContext on gpsimd so the tensor
    # engine's HAM warmup can start right after the framework barrier
    # (the in-block vector memset otherwise gates it until ~7.9us).  No
    # sem guard: gpsimd finishes the memset ~0.7us before the tensor
    # engine clears its block prologue, and warmup DATA is discarded
    # anyway.
    wsrc_t = nc.alloc_sbuf_tensor("wsrc", [128, 2, 256], mybir.dt.float8e4)
    nc.gpsimd.memset(wsrc_t.ap(), 0.25)
    wsrc = wsrc_t.ap()
    xpan_d = nc.declare_dram_parameter("xpan", [128, 5, 2, 2, PW], f8, isOutput=False)
    xlhs_d = nc.declare_dram_parameter("xlhs", [128, 4, 2, 2, 128], f8, isOutput=False)
    xd4_d = nc.declare_dram_parameter("xd4", [128, 2, 2, 2, 128], f8, isOutput=False)
    sq8_d = nc.declare_dram_parameter("sq8", [4, 2, 5, PW], f8, isOutput=False)
    one8_d = nc.declare_dram_parameter("one8", [4, 2, 128], f8, isOutput=False)
    sqb_d = nc.declare_dram_parameter("sqb", [128, NSLOT], f32, isOutput=False)
    uu_d = nc.declare_dram_parameter("uu", [128, NSLOT, 2, 32], f8, isOutput=False)
    out_d = nc.declare_dram_parameter("out", [32, NGRP, PW], f32, isOutput=True)

    with tile.TileContext(nc) as tc, ExitStack() as ctx:
        singles = ctx.enter_context(tc.tile_pool(name="singles", bufs=1))
        work = ctx.enter_context(tc.tile_pool(name="work", bufs=6))
        pd2 = ctx.enter_context(tc.tile_pool(name="pd2", bufs=5, space="PSUM"))
        pT = ctx.enter_context(tc.tile_pool(name="pT", bufs=2, space="PSUM"))

        # --- input DMAs: XP0 alone in front of the scalar queue so its
        # transfer starts ASAP; tiny consts on sync.  HW DMA engines drain
        # competing queues unfairly, so nothing sizable may precede XP0.
        XP = singles.tile([128, 5, 2, 2, PW], f8)
        XLH = singles.tile([128, 4, 2, 2, 128], f8)
        nc.scalar.dma_start(out=XP[:, 0], in_=xpan_d[:, 0])
        nc.sync.dma_start(out=XLH, in_=xlhs_d[:, :, :, :, :])
        ONE8 = singles.tile([4, 2, 128], f8)
        SQ8 = singles.tile([4, 2, 5, PW], f8)
        SQB = singles.tile([128, NSLOT], f32)
        UU = singles.tile([128, NSLOT, 2, 32], f8)
        XD4 = singles.tile([128, 2, 2, 2, 128], f8)
        nc.sync.dma_start(out=ONE8, in_=one8_d[:, :, :])
        nc.sync.dma_start(out=SQ8, in_=sq8_d[:, :, :, :])
        nc.sync.dma_start(out=SQB, in_=sqb_d[:, :])
        nc.sync.dma_start(out=UU[:, 0:4], in_=uu_d[:, 0:4])

        # HAM warmup: a dense back-to-back matmul burst earns the PE's
        # full-speed grant (~3.4us of ~80%+ duty required); it must run
        # gapless into the real stream or the grant is forfeited.
        pwarm = ctx.enter_context(tc.tile_pool(name="pwarm", bufs=1, space="PSUM"))
        wp = pwarm.tile([128, 256], mybir.dt.float32, name="wp")
        for _ in range(WARMUP):
            nc.tensor.matmul(wp, wsrc[:, :, 0:128], wsrc,
                             start=True, stop=True,
                             perf_mode=mybir.MatmulPerfMode.DoubleRow)

        # Preload the sqrt activation table during the DMA preamble.
        warm = singles.tile([1, 2], f32)
        nc.vector.memset(warm, 1.0)
        nc.scalar.activation(out=warm, in_=warm,
                             func=mybir.ActivationFunctionType.Sqrt)

        nc.scalar.dma_start(out=XP[:, 1], in_=xpan_d[:, 1])
        nc.scalar.dma_start(out=XP[:, 2], in_=xpan_d[:, 2])
        nc.sync.dma_start(out=UU[:, 4:NSLOT], in_=uu_d[:, 4:NSLOT])
        nc.sync.dma_start(out=XD4, in_=xd4_d[:, :, :, :, :])
        nc.sync.dma_start(out=XP[:, 3], in_=xpan_d[:, 3])
        nc.sync.dma_start(out=XP[:, 4], in_=xpan_d[:, 4])

        DELAY = 4
        Ttiles = {}
        dq_tiles = {}

        def lhs_ap(t):
            src, k = LHS[t]
            if src == 0:
                return (XLH[:, k, 0], XLH[:, k, 1])
            return (XD4[:, k, 0], XD4[:, k, 1])

        def dist_stage(t):
            g = RSLOT[t]
            la0, la1 = lhs_ap(t)
            d2 = pd2.tile([128, PW], mybir.dt.float32, name="d2")
            nc.tensor.matmul(d2, la0, XP[:, g, 0],
                             start=True, stop=False, perf_mode=DR)
            nc.tensor.matmul(d2, la1, XP[:, g, 1],
                             start=False, stop=False, perf_mode=DR)
            nc.tensor.matmul(d2, ONE8, SQ8[:, :, g],
                             start=False, stop=True, perf_mode=DR)
            db = work.tile([128, PW], mybir.dt.bfloat16, name="db")
            nc.scalar.activation(out=db, in_=d2,
                                 func=mybir.ActivationFunctionType.Sqrt,
                                 bias=SQB[:, t:t + 1], scale=-2.0)
            dq = work.tile([128, 2, PW], mybir.dt.float8e4, name="dq")
            nc.vector.tensor_scalar(out=dq[:, 0, :], in0=db,
                                    scalar1=float(SHIFT), scalar2=None,
                                    op0=mybir.AluOpType.subtract)
            nc.vector.tensor_scalar(out=dq[:, 1, :], in0=db,
                                    scalar1=1.0, scalar2=0.0,
                                    op0=mybir.AluOpType.subtract,
                                    op1=mybir.AluOpType.min)
            dq_tiles[t] = dq

        def grp(t):
            for g in range(NGRP):
                if GSTART[g] <= t < GSTART[g] + GSIZE[g]:
                    return g, t - GSTART[g]
            raise AssertionError

        def stats_stage(t):
            g, k = grp(t)
            if k == 0:
                Ttiles[g] = pT.tile([32, PW], mybir.dt.float32, name="T")
            nc.tensor.matmul(Ttiles[g], UU[:, t], dq_tiles.pop(t),
                             start=(k == 0), stop=(k == GSIZE[g] - 1),
                             perf_mode=DR)
            if k == GSIZE[g] - 1:
                Tsb = work.tile([32, PW], mybir.dt.float32, name="Tsb")
                nc.scalar.copy(out=Tsb, in_=Ttiles.pop(g))
                nc.sync.dma_start(out=out_d[:, g], in_=Tsb)

        for t in range(NSLOT):
            dist_stage(t)
            if t >= DELAY:
                stats_stage(t - DELAY)
        for t in range(NSLOT - DELAY, NSLOT):
            stats_stage(t)

    _split_waits(nc)
    _NC_CACHE["nc"] = nc
    return nc


def _masks(y, ds):
    cc = (np.arange(12) // 3)[None, :]
    aa = (np.arange(12) % 3)[None, :]
    U_sa = ((y[:, None] == cc) & (ds[:, None] < aa))
    U_s = ((y[:, None] < cc) & (ds[:, None] < aa))
    U_sa_sym = ((y[:, None] == cc) & (ds[:, None] != aa))
    U_s_sym = U_s | ((y[:, None] > cc) & (ds[:, None] > aa))
    return U_sa, U_s, U_sa_sym, U_s_sym


def prepare_inputs(X, ds, y):
    X = np.asarray(X, dtype=np.float32)
    ds = np.asarray(ds).astype(np.int64)
    y = np.asarray(y).astype(np.int64)

    X8 = X.astype(F8)
    Xd = X8.astype(np.float64)
    sq = (Xd * Xd).sum(axis=1)
    sq32 = sq.astype(np.float32)
    # -sq_j/2 enters PSUM as 4*(hi+lo+lo2) with fp8 terms (|hi| ~ 64 stays
    # well inside float8_e4m3's +-240 range; the rank-4 matmul's ONE8 = 4).
    t64 = (-sq32.astype(np.float64) / 8)
    hi = t64.astype(F8)
    r = t64 - hi.astype(np.float64)
    lo = r.astype(F8)
    lo2 = (r - lo.astype(np.float64)).astype(F8)

    # [a(256-half), b(dr), c(128), col]
    XT8 = np.ascontiguousarray(X8.T.reshape(2, 2, 128, BS))

    U_sa, U_s, U_sa_sym, U_s_sym = _masks(y, ds)

    one8 = np.full((4, 2, 128), 4.0, dtype=F8)

    in_maps = []
    for c in range(NCORES):
        tiles = tiles_for_core(c)
        xpan = np.empty((128, 5, 2, 2, PW), dtype=F8)
        sq8 = np.zeros((4, 2, 5, PW), dtype=F8)
        for s in range(5):
            p = tiles[GSTART[s]][1]
            jsl = slice(p * PW, (p + 1) * PW)
            xpan[:, s] = XT8[:, :, :, jsl].transpose(2, 0, 1, 3)
            sq8[0, 0, s] = hi[jsl]
            sq8[0, 1, s] = lo[jsl]
            sq8[1, 0, s] = lo2[jsl]
        # xlhs: own-band chunks in contiguous [c, chunk, a, b, 128] layout
        xlhs = np.empty((128, 4, 2, 2, 128), dtype=F8)
        for k in range(4):
            rs = tiles[k][0]
            xlhs[:, k] = XT8[:, :, :, rs:rs + 128].transpose(2, 0, 1, 3)
        # xd4: lhs chunks for the two G4 tiles, same layout
        xd4 = np.empty((128, 2, 2, 2, 128), dtype=F8)
        for k in range(2):
            rs = tiles[16 + k][0]
            xd4[:, k] = XT8[:, :, :, rs:rs + 128].transpose(2, 0, 1, 3)

        sqb = np.empty((128, NSLOT), dtype=np.float32)
        uu = np.zeros((128, NSLOT, 2, 32), dtype=F8)
        for t, (rs, p, ordered) in enumerate(tiles):
            isl = slice(rs, rs + 128)
            sqb[:, t] = sq32[isl] + np.float32(C0)
            Ua = U_sa if ordered else U_sa_sym
            Us = U_s if ordered else U_s_sym
            uu[:, t, 0, 0:12] = Ua[isl].astype(F8)
            uu[:, t, 1, 12:24] = Us[isl].astype(F8)
        in_maps.append({
            "xpan": xpan, "xlhs": xlhs, "xd4": xd4, "sq8": sq8,
            "one8": one8, "sqb": sqb, "uu": uu,
        })
    return in_maps


def finish(results, ds, y, n_classes, n_domains):
    ds = np.asarray(ds).astype(np.int64)
    y = np.asarray(y).astype(np.int64)
    n_classes = int(n_classes)
    n_domains = int(n_domains)
    combo = (y * 3 + ds).astype(np.int64)

    U_sa, U_s, U_sa_sym, U_s_sym = _masks(y, ds)
    jloc = np.arange(PW)

    sa_sum = 0.0
    s_hinge = 0.0
    for c in range(NCORES):
        tiles = tiles_for_core(c)
        T = np.asarray(results[c]["out"], dtype=np.float64)  # (32, NGRP, PW)
        for g in range(NGRP):
            p = tiles[GSTART[g]][1]
            jsl = slice(p * PW, (p + 1) * PW)
            combo_p = combo[jsl]
            nloc = np.bincount(combo_p, minlength=12).astype(np.int64)
            sa_sum += T[0:12, g][combo_p, jloc].sum()
            s_hinge -= T[12:24, g][combo_p, jloc].sum()
            for k in range(GSIZE[g]):
                rs, _, ordered = tiles[GSTART[g] + k]
                isl = slice(rs, rs + 128)
                Ua = U_sa if ordered else U_sa_sym
                cnt = Ua[isl].sum(axis=0).astype(np.int64)
                sa_sum += SHIFT * float(cnt @ nloc)

    n_sa = n_classes * (n_domains * (n_domains - 1) // 2)
    n_s = (n_classes * (n_classes - 1) // 2) * (n_domains * (n_domains - 1) // 2)
    sa_loss = 0.5 * sa_sum / n_sa
    s_loss = 0.5 * s_hinge / n_s
    return np.array([sa_loss, s_loss], dtype=np.float32)


def run_device(in_maps, trace=False, **kw):
    nc = build_program()
    return run_bass_kernel_spmd(nc, in_maps, core_ids=list(range(NCORES)),
                                trace=trace, **kw)


def kernel(X, ds, y, n_classes, n_domains):
    in_maps = prepare_inputs(X, ds, y)
    res = run_device(in_maps)
    return finish(res.results, ds, y, n_classes, n_domains)


# revision 7
# speedup vs baseline: 1.0169x; 1.0169x over previous
"""JointCCSA loss kernel — v4: circulant band tiling, shared-X operands.

Each core c owns row-band c (512 rows) and covers pair-blocks (c, c+d) for
d=0..3 plus half of (c, c+4): 18 real [128x512] tiles, no dummy slots.
lhs and rhs both read from ONE shared fp8 X-panel buffer (plain X, not -2X);
the -2 factor moves into the ACT sqrt's scale.  sq_j enters the PSUM via a
rank-4 fp8 DoubleRow matmul (hi/lo/lo2 split), sq_i via the ACT bias.
Per-core input drops 2.35MB -> ~1.45MB and is DMA'd in need-order so the
first group (self-band, needs one 256KB panel) starts at ~6.5us.
Postamble semaphore sweep shrunk via walrus --max-sem-num.
"""

import numpy as np
import ml_dtypes
from contextlib import ExitStack

import concourse.bass as bass
import concourse.tile as tile
from concourse import mybir
from concourse.vector_clock import ScopedClock
from concourse.bass_utils import run_bass_kernel_spmd
import concourse.bass_utils as _bass_utils

BS = 4096
D = 512
NCORES = 8
PW = 512
NSLOT = 18
NGRP = 5
GSIZE = [4, 4, 4, 4, 2]
GSTART = [0, 4, 8, 12, 16]
C0 = 0.0625
SHIFT = 32.0
WARMUP = 18
F8 = ml_dtypes.float8_e4m3
BF16 = ml_dtypes.bfloat16

# --- walrus build patches (same as v3) -------------------------------------
def _patched_drain_and_barrier(self, tick_clock, wait_clock):
    nc = self.nc
    coll = nc.sync.nop(nofuse=True, hint="drain_wait_collector")
    wait_clock.add_sem_waits(coll.ins, ScopedClock({None: tick_clock.global_clock}))
    si = coll.ins.sync_info
    waits = list(si.on_wait) if si is not None else []
    if len(waits) > 1:
        si.on_wait = [waits[0]]
        for w in waits[1:]:
            n = nc.sync.nop(nofuse=True, hint="drain_wait_extra")
            n.ins.sync_info = mybir.SyncInfo(on_wait=[w], on_update=[])
    nc.sync.drain()
    nc.all_engine_barrier()
    assert self.sems is not None
    popped = nc._tile_sem_poison_stack.pop()
    assert popped is self._sem_poison
    nc.clear_and_free_semaphores(list(self.sems.allocated().values()))
    nc.all_engine_barrier()


tile.TileContext._drain_and_barrier = _patched_drain_and_barrier


def _split_waits(nc, maxw=1):
    for fn in nc.m.functions:
        for blk in fn.blocks:
            newlist = []
            for inst in blk.instructions:
                si = getattr(inst, "sync_info", None)
                if si is not None and len(si.on_wait) > maxw:
                    waits = list(si.on_wait)
                    for i, w in enumerate(waits[maxw:]):
                        nop = mybir.InstNoOp(
                            name=f"{inst.name}-wsplit{i}",
                            sync_info=mybir.SyncInfo(on_wait=[w], on_update=[]),
                            bass_nofuse=True,
                            engine=inst.engine,
                        )
                        nc.register_instruction(nop)
                        newlist.append(nop)
                    si.on_wait = waits[:maxw]
                newlist.append(inst)
            blk.instructions[:] = newlist
# ---------------------------------------------------------------------------


def tiles_for_core(c):
    """18 tiles of (rows_start, panel, ordered), groups (4,4,4,4,2)."""
    out = []
    for d in range(4):
        p = (c + d) % 8
        for k in range(4):
            out.append((c * 512 + k * 128, p, d == 0))
    if c < 4:
        for k in range(2):
            out.append((c * 512 + k * 128, c + 4, False))
    else:
        for k in range(2):
            out.append(((c - 4) * 512 + (2 + k) * 128, c, False))
    return out


# slot -> (lhs source, lhs col offset).  lhs source 0 = xpan slot 0 (own
# band), 1 = xd4.  rhs panel-slot for slot t: RSLOT[t].
LHS = [(0, (k % 4) * 128) for k in range(16)] + [(1, 0), (1, 128)]
RSLOT = [0] * 4 + [1] * 4 + [2] * 4 + [3] * 4 + [4] * 2

_NC_CACHE = {}


def build_program():
    if "nc" in _NC_CACHE:
        return _NC_CACHE["nc"]
    f32 = mybir.dt.float32
    bf16 = mybir.dt.bfloat16
    f8 = mybir.dt.float8e4
    DR = mybir.MatmulPerfMode.DoubleRow

    nc = bass.Bass()
    # Warmup source initialized pre-TileContext on gpsimd so the tensor
    # engine's HAM warmup can start right after the framework barrier
    # (the in-block vector memset otherwise gates it until ~7.9us).  No
    # sem guard: gpsimd finishes the memset ~0.7us before the tensor
    # engine clears its block prologue, and warmup DATA is discarded
    # anyway.
    wsrc_t = nc.alloc_sbuf_tensor("wsrc", [128, 2, 256], mybir.dt.float8e4)
    nc.gpsimd.memset(wsrc_t.ap(), 0.25)
    wsrc = wsrc_t.ap()
    xpan_d = nc.declare_dram_parameter("xpan", [128, 5, 2, 2, PW], f8, isOutput=False)
    xd4_d = nc.declare_dram_parameter("xd4", [128, 2, 2, 256], f8, isOutput=False)
    sq8_d = nc.declare_dram_parameter("sq8", [4, 2, 5, PW], f8, isOutput=False)
    one8_d = nc.declare_dram_parameter("one8", [4, 2, 128], f8, isOutput=False)
    sqb_d = nc.declare_dram_parameter("sqb", [128, NSLOT], f32, isOutput=False)
    uu_d = nc.declare_dram_parameter("uu", [128, NSLOT, 2, 32], f8, isOutput=False)
    out_d = nc.declare_dram_parameter("out", [32, NGRP, PW], f32, isOutput=True)

    with tile.TileContext(nc) as tc, ExitStack() as ctx:
        singles = ctx.enter_context(tc.tile_pool(name="singles", bufs=1))
        work = ctx.enter_context(tc.tile_pool(name="work", bufs=6))
        pd2 = ctx.enter_context(tc.tile_pool(name="pd2", bufs=5, space="PSUM"))
        pT = ctx.enter_context(tc.tile_pool(name="pT", bufs=2, space="PSUM"))

        # --- input DMAs: XP0 alone in front of the scalar queue so its
        # transfer starts ASAP; tiny consts on sync.  HW DMA engines drain
        # competing queues unfairly, so nothing sizable may precede XP0.
        XP = singles.tile([128, 5, 2, 2, PW], f8)
        nc.scalar.dma_start(out=XP[:, 0], in_=xpan_d[:, 0])
        ONE8 = singles.tile([4, 2, 128], f8)
        SQ8 = singles.tile([4, 2, 5, PW], f8)
        SQB = singles.tile([128, NSLOT], f32)
        UU = singles.tile([128, NSLOT, 2, 32], f8)
        XD4 = singles.tile([128, 2, 2, 256], f8)
        nc.sync.dma_start(out=ONE8, in_=one8_d[:, :, :])
        nc.sync.dma_start(out=SQ8, in_=sq8_d[:, :, :, :])
        nc.sync.dma_start(out=SQB, in_=sqb_d[:, :])
        nc.sync.dma_start(out=UU[:, 0:4], in_=uu_d[:, 0:4])

        # HAM warmup: a dense back-to-back matmul burst earns the PE's
        # full-speed grant (~3.4us of ~80%+ duty required); it must run
        # gapless into the real stream or the grant is forfeited.
        pwarm = ctx.enter_context(tc.tile_pool(name="pwarm", bufs=1, space="PSUM"))
        wp = pwarm.tile([128, 256], mybir.dt.float32, name="wp")
        for _ in range(WARMUP):
            nc.tensor.matmul(wp, wsrc[:, :, 0:128], wsrc,
                             start=True, stop=True,
                             perf_mode=mybir.MatmulPerfMode.DoubleRow)

        # Preload the sqrt activation table during the DMA preamble.
        warm = singles.tile([1, 2], f32)
        nc.vector.memset(warm, 1.0)
        nc.scalar.activation(out=warm, in_=warm,
                             func=mybir.ActivationFunctionType.Sqrt)

        nc.scalar.dma_start(out=XP[:, 1], in_=xpan_d[:, 1])
        nc.scalar.dma_start(out=XP[:, 2], in_=xpan_d[:, 2])
        nc.sync.dma_start(out=UU[:, 4:NSLOT], in_=uu_d[:, 4:NSLOT])
        nc.sync.dma_start(out=XD4, in_=xd4_d[:, :, :, :])
        nc.sync.dma_start(out=XP[:, 3], in_=xpan_d[:, 3])
        nc.sync.dma_start(out=XP[:, 4], in_=xpan_d[:, 4])

        DELAY = 4
        Ttiles = {}
        dq_tiles = {}

        def lhs_ap(t):
            src, off = LHS[t]
            if src == 0:
                return (XP[:, 0, 0, :, off:off + 128],
                        XP[:, 0, 1, :, off:off + 128])
            return (XD4[:, 0, :, off:off + 128],
                    XD4[:, 1, :, off:off + 128])

        def dist_stage(t):
            g = RSLOT[t]
            la0, la1 = lhs_ap(t)
            d2 = pd2.tile([128, PW], mybir.dt.float32, name="d2")
            nc.tensor.matmul(d2, la0, XP[:, g, 0],
                             start=True, stop=False, perf_mode=DR)
            nc.tensor.matmul(d2, la1, XP[:, g, 1],
                             start=False, stop=False, perf_mode=DR)
            nc.tensor.matmul(d2, ONE8, SQ8[:, :, g],
                             start=False, stop=True, perf_mode=DR)
            db = work.tile([128, PW], mybir.dt.bfloat16, name="db")
            nc.scalar.activation(out=db, in_=d2,
                                 func=mybir.ActivationFunctionType.Sqrt,
                                 bias=SQB[:, t:t + 1], scale=-2.0)
            dq = work.tile([128, 2, PW], mybir.dt.float8e4, name="dq")
            nc.vector.tensor_scalar(out=dq[:, 0, :], in0=db,
                                    scalar1=float(SHIFT), scalar2=None,
                                    op0=mybir.AluOpType.subtract)
            nc.vector.tensor_scalar(out=dq[:, 1, :], in0=db,
                                    scalar1=1.0, scalar2=0.0,
                                    op0=mybir.AluOpType.subtract,
                                    op1=mybir.AluOpType.min)
            dq_tiles[t] = dq

        def grp(t):
            for g in range(NGRP):
                if GSTART[g] <= t < GSTART[g] + GSIZE[g]:
                    return g, t - GSTART[g]
            raise AssertionError

        def stats_stage(t):
            g, k = grp(t)
            if k == 0:
                Ttiles[g] = pT.tile([32, PW], mybir.dt.float32, name="T")
            nc.tensor.matmul(Ttiles[g], UU[:, t], dq_tiles.pop(t),
                             start=(k == 0), stop=(k == GSIZE[g] - 1),
                             perf_mode=DR)
            if k == GSIZE[g] - 1:
                Tsb = work.tile([32, PW], mybir.dt.float32, name="Tsb")
                nc.scalar.copy(out=Tsb, in_=Ttiles.pop(g))
                nc.sync.dma_start(out=out_d[:, g], in_=Tsb)

        for t in range(NSLOT):
            dist_stage(t)
            if t >= DELAY:
                stats_stage(t - DELAY)
        for t in range(NSLOT - DELAY, NSLOT):
            stats_stage(t)

    _split_waits(nc)
    _NC_CACHE["nc"] = nc
    return nc


def _masks(y, ds):
    cc = (np.arange(12) // 3)[None, :]
    aa = (np.arange(12) % 3)[None, :]
    U_sa = ((y[:, None] == cc) & (ds[:, None] < aa))
    U_s = ((y[:, None] < cc) & (ds[:, None] < aa))
    U_sa_sym = ((y[:, None] == cc) & (ds[:, None] != aa))
    U_s_sym = U_s | ((y[:, None] > cc) & (ds[:, None] > aa))
    return U_sa, U_s, U_sa_sym, U_s_sym


def prepare_inputs(X, ds, y):
    X = np.asarray(X, dtype=np.float32)
    ds = np.asarray(ds).astype(np.int64)
    y = np.asarray(y).astype(np.int64)

    X8 = X.astype(F8)
    Xd = X8.astype(np.float64)
    sq = (Xd * Xd).sum(axis=1)
    sq32 = sq.astype(np.float32)
    # -sq_j/2 enters PSUM as 4*(hi+lo+lo2) with fp8 terms (|hi| ~ 64 stays
    # well inside float8_e4m3's +-240 range; the rank-4 matmul's ONE8 = 4).
    t64 = (-sq32.astype(np.float64) / 8)
    hi = t64.astype(F8)
    r = t64 - hi.astype(np.float64)
    lo = r.astype(F8)
    lo2 = (r - lo.astype(np.float64)).astype(F8)

    # [a(256-half), b(dr), c(128), col]
    XT8 = np.ascontiguousarray(X8.T.reshape(2, 2, 128, BS))

    U_sa, U_s, U_sa_sym, U_s_sym = _masks(y, ds)

    one8 = np.full((4, 2, 128), 4.0, dtype=F8)

    in_maps = []
    for c in range(NCORES):
        tiles = tiles_for_core(c)
        xpan = np.empty((128, 5, 2, 2, PW), dtype=F8)
        sq8 = np.zeros((4, 2, 5, PW), dtype=F8)
        for s in range(5):
            p = tiles[GSTART[s]][1]
            jsl = slice(p * PW, (p + 1) * PW)
            xpan[:, s] = XT8[:, :, :, jsl].transpose(2, 0, 1, 3)
            sq8[0, 0, s] = hi[jsl]
            sq8[0, 1, s] = lo[jsl]
            sq8[1, 0, s] = lo2[jsl]
        # xd4: lhs columns for the two G4 tiles (rows as X^T cols)
        r0 = tiles[16][0]
        isl = slice(r0, r0 + 256)
        xd4 = np.ascontiguousarray(
            XT8[:, :, :, isl].transpose(2, 0, 1, 3))  # [128,2,2,256]

        sqb = np.empty((128, NSLOT), dtype=np.float32)
        uu = np.zeros((128, NSLOT, 2, 32), dtype=F8)
        for t, (rs, p, ordered) in enumerate(tiles):
            isl = slice(rs, rs + 128)
            sqb[:, t] = sq32[isl] + np.float32(C0)
            Ua = U_sa if ordered else U_sa_sym
            Us = U_s if ordered else U_s_sym
            uu[:, t, 0, 0:12] = Ua[isl].astype(F8)
            uu[:, t, 1, 12:24] = Us[isl].astype(F8)
        in_maps.append({
            "xpan": xpan, "xd4": xd4, "sq8": sq8, "one8": one8,
            "sqb": sqb, "uu": uu,
        })
    return in_maps


def finish(results, ds, y, n_classes, n_domains):
    ds = np.asarray(ds).astype(np.int64)
    y = np.asarray(y).astype(np.int64)
    n_classes = int(n_classes)
    n_domains = int(n_domains)
    combo = (y * 3 + ds).astype(np.int64)

    U_sa, U_s, U_sa_sym, U_s_sym = _masks(y, ds)
    jloc = np.arange(PW)

    sa_sum = 0.0
    s_hinge = 0.0
    for c in range(NCORES):
        tiles = tiles_for_core(c)
        T = np.asarray(results[c]["out"], dtype=np.float64)  # (32, NGRP, PW)
        for g in range(NGRP):
            p = tiles[GSTART[g]][1]
            jsl = slice(p * PW, (p + 1) * PW)
            combo_p = combo[jsl]
            nloc = np.bincount(combo_p, minlength=12).astype(np.int64)
            sa_sum += T[0:12, g][combo_p, jloc].sum()
            s_hinge -= T[12:24, g][combo_p, jloc].sum()
            for k in range(GSIZE[g]):
                rs, _, ordered = tiles[GSTART[g] + k]
                isl = slice(rs, rs + 128)
                Ua = U_sa if ordered else U_sa_sym
                cnt = Ua[isl].sum(axis=0).astype(np.int64)
                sa_sum += SHIFT * float(cnt @ nloc)

    n_sa = n_classes * (n_domains * (n_domains - 1) // 2)
    n_s = (n_classes * (n_classes - 1) // 2) * (n_domains * (n_domains - 1) // 2)
    sa_loss = 0.5 * sa_sum / n_sa
    s_loss = 0.5 * s_hinge / n_s
    return np.array([sa_loss, s_loss], dtype=np.float32)


def run_device(in_maps, trace=False, **kw):
    nc = build_program()
    return run_bass_kernel_spmd(nc, in_maps, core_ids=list(range(NCORES)),
                                trace=trace, **kw)


def kernel(X, ds, y, n_classes, n_domains):
    in_maps = prepare_inputs(X, ds, y)
    res = run_device(in_maps)
    return finish(res.results, ds, y, n_classes, n_domains)
